# revision 1
# baseline (speedup 1.0000x reference)
"""HGNN conv kernel for Trainium2, 8 NeuronCores.

out = dv ⊙ (H @ (W·de ⊙ (H^T @ (dv ⊙ (x@weight))))) + bias
  dv = rowsum(H)^-1/2  [N], de = colsum(H)^-1  [E]
  N=16384, E=8192, F=64.

Sharding: H/x row-sharded over N across 8 cores (2048 rows each).
Host preps per-core bf16 H shard in both layouts (natural + transposed) —
a pure layout/precision transform; all FLOPs (matmuls, reductions,
scalings) run on device.

Device per core:
  pass 1: stream H natural [128,8192] row-tiles; DVE row-sums -> dv;
          xw = x@weight (PE); xs' = [dv*xw | 1] bf16 stationary;
          y^T[65,512-blk] += xs'^T @ H  (PSUM accum over 4-tile n-groups,
          DVE flush to f32 SBUF acc). Ones column yields colsum partials.
  AllReduce [65,8192] f32 across 8 cores.
  y2 = (W*de) * y_sum  via PE transpose + ACT scaled copy -> bf16 [e,64].
  pass 2: stream H^T [128e,512n] tiles; out^T[64,512] += y2^T @ H^T;
          PE transpose back, ACT copy scaled by dv, DVE bias add, DMA out.
"""

import numpy as np
import ml_dtypes

N, E, F = 16384, 8192, 64
NCORES = 8
NL = N // NCORES          # 2048 rows per core
P = 128
NT = NL // P              # 16 n-tiles per core
ET = E // P               # 64 e-tiles
EBLK = 512
EB = E // EBLK            # 16 e-blocks (pass 1 moving free dim)
NBLK = 512
NB = NL // NBLK           # 4 n-blocks (pass 2 moving free dim)
G = 4                     # n-tiles per PSUM accumulation group (pass 1)

_prog_cache = {}


def _build_program():
    import concourse.bass as bass
    import concourse.mybir as mybir
    import concourse.tile as tile
    from concourse import bacc
    from concourse.masks import make_identity

    f32 = mybir.dt.float32
    bf16 = mybir.dt.bfloat16
    Copy = mybir.ActivationFunctionType.Copy
    add = mybir.AluOpType.add
    mult = mybir.AluOpType.mult
    X = mybir.AxisListType.X

    nc = bacc.Bacc(
        "TRN2", target_bir_lowering=False, debug=False, num_devices=NCORES
    )
    h = nc.declare_dram_parameter("h", [NL, E], bf16, isOutput=False)
    ht = nc.declare_dram_parameter("ht", [E, NL], bf16, isOutput=False)
    xt = nc.declare_dram_parameter("xt", [F, NL], f32, isOutput=False)
    wmat = nc.declare_dram_parameter("wmat", [F, F], f32, isOutput=False)
    wstr = nc.declare_dram_parameter("wstr", [P, ET], f32, isOutput=False)
    biasb = nc.declare_dram_parameter("biasb", [P, F], f32, isOutput=False)
    out = nc.declare_dram_parameter("out", [NL, F], f32, isOutput=True)

    with tile.TileContext(nc) as tc:
        with (
            tc.tile_pool(name="hp", bufs=G + 1) as hp,           # H row tiles
            tc.tile_pool(name="xsp", bufs=G + 1) as xsp,         # xs' tiles
            tc.tile_pool(name="accp", bufs=1) as accp,           # y acc
            tc.tile_pool(name="smallp", bufs=1) as smallp,       # persistent small
            tc.tile_pool(name="rp", bufs=4) as rp,               # rowsum temps
            tc.tile_pool(name="htp", bufs=8) as htp,             # HT row tiles
            tc.tile_pool(name="outp", bufs=4) as outp,           # out staging
            tc.tile_pool(name="ps_small", bufs=2, space="PSUM") as ps_small,
            tc.tile_pool(name="ps_big", bufs=2, space="PSUM") as ps_big,
            tc.tile_pool(name="ps2", bufs=1, space="PSUM") as ps2,
            tc.tile_pool(name="dramp", bufs=1, space="DRAM") as dramp,
        ):
            # ---- persistent small tensors ----
            xt_sb = smallp.tile([F, NL], f32, tag="xt")
            nc.sync.dma_start(xt_sb[:], xt[:, :])
            wmat_sb = smallp.tile([F, F], f32, tag="wmat")
            nc.sync.dma_start(wmat_sb[:], wmat[:, :])
            wstr_sb = smallp.tile([P, ET], f32, tag="wstr")
            nc.sync.dma_start(wstr_sb[:], wstr[:, :])
            bias_sb = smallp.tile([P, F], f32, tag="bias")
            nc.sync.dma_start(bias_sb[:], biasb[:, :])
            ident = smallp.tile([F, F], f32, tag="ident")
            make_identity(nc, ident)
            dv_all = smallp.tile([P, NT], f32, tag="dv")
            y_acc = accp.tile([F + 1, E], f32, tag="yacc")

            # ---- pass 1: y^T[f,e] (+ colsum row) over n-groups ----
            for g in range(NT // G):
                group = []
                for i in range(G):
                    t = g * G + i
                    h_t = hp.tile([P, E], bf16, tag="h")
                    nc.sync.dma_start(h_t[:], h[t * P:(t + 1) * P, :])
                    # rowsum -> dv = sqrt(1/rowsum); split across DVE and ACT
                    rsum = rp.tile([P, 1], f32, tag="rsum")
                    if i % 2 == 0:
                        nc.vector.tensor_reduce(
                            out=rsum[:], in_=h_t[:], axis=X, op=add
                        )
                    else:
                        # in-place copy on ScalarE; accum_out gives the row sum
                        nc.scalar.activation(
                            out=h_t[:], in_=h_t[:], func=Copy, accum_out=rsum[:]
                        )
                    rinv = rp.tile([P, 1], f32, tag="rinv")
                    nc.vector.reciprocal(out=rinv[:], in_=rsum[:])
                    nc.scalar.sqrt(out=dv_all[:, t:t + 1], in_=rinv[:])
                    # xw = x @ weight for this tile
                    xw_ps = ps_small.tile([P, F], f32, tag="tp")
                    nc.tensor.matmul(
                        xw_ps[:], lhsT=xt_sb[:, t * P:(t + 1) * P], rhs=wmat_sb[:],
                        start=True, stop=True,
                    )
                    xs_t = xsp.tile([P, F + 1], bf16, tag="xs")
                    nc.scalar.activation(
                        out=xs_t[:, 0:F], in_=xw_ps[:], func=Copy,
                        scale=dv_all[:, t:t + 1],
                    )
                    nc.gpsimd.memset(xs_t[:, F:F + 1], 1.0)
                    group.append((xs_t, h_t))
                for b in range(EB):
                    yps = ps_big.tile([F + 1, EBLK], f32, tag="yps")
                    for i, (xs_t, h_t) in enumerate(group):
                        nc.tensor.matmul(
                            yps[:], lhsT=xs_t[:], rhs=h_t[:, b * EBLK:(b + 1) * EBLK],
                            start=(i == 0), stop=(i == G - 1),
                        )
                    dst = y_acc[:, b * EBLK:(b + 1) * EBLK]
                    if g == 0:
                        nc.vector.tensor_copy(out=dst, in_=yps[:])
                    else:
                        nc.vector.tensor_tensor(out=dst, in0=dst, in1=yps[:], op=add)

            # ---- AllReduce in 2 halves so pass 2 can start on half 0 ----
            EH = E // 2
            ETH = ET // 2
            y2_sb = smallp.tile([P, ET, F], bf16, tag="y2")
            for hf in range(2):
                b_in = dramp.tile([F + 1, EH], f32, name=f"bi{hf}")
                b_out = dramp.tile([F + 1, EH], f32, name=f"bo{hf}")
                nc.sync.dma_start(b_in[:], y_acc[:, hf * EH:(hf + 1) * EH])
                nc.gpsimd.collective_compute(
                    "AllReduce",
                    mybir.AluOpType.add,
                    ins=[b_in[:].opt()],
                    outs=[b_out[:].opt()],
                    replica_groups=[list(range(NCORES))],
                )
                # y2 = (W * de) * y_sum for this half, transposed to [e,64].
                # Reduced rows overwrite the local partial in y_acc (saves SBUF).
                nc.sync.dma_start(y_acc[0:F, hf * EH:(hf + 1) * EH], b_out[0:F, :])
                cs = smallp.tile([P, ETH], f32, name=f"cs{hf}")
                nc.sync.dma_start(
                    cs[:], b_out[F, :].rearrange("(o p) -> p o", p=P)
                )
                de_t = smallp.tile([P, ETH], f32, name=f"de{hf}")
                nc.vector.reciprocal(out=de_t[:], in_=cs[:])
                wde = smallp.tile([P, ETH], f32, name=f"wde{hf}")
                nc.vector.tensor_tensor(
                    out=wde[:], in0=de_t[:],
                    in1=wstr_sb[:, hf * ETH:(hf + 1) * ETH], op=mult,
                )
                for tt in range(ETH):
                    t = hf * ETH + tt
                    tp = ps_small.tile([P, F], f32, tag="tp")
                    nc.tensor.transpose(
                        tp[:], y_acc[0:F, t * P:(t + 1) * P], ident[:]
                    )
                    nc.scalar.activation(
                        out=y2_sb[:, t, :], in_=tp[:], func=Copy,
                        scale=wde[:, tt:tt + 1],
                    )

            # ---- pass 2: t-outer; 4 persistent PSUM banks; big HT DMAs ----
            o_tiles = [ps2.tile([F, NBLK], f32, name=f"o{j}") for j in range(NB)]
            for t in range(ET):
                htt = htp.tile([P, NL], bf16, tag="ht")
                nc.sync.dma_start(htt[:], ht[t * P:(t + 1) * P, :])
                for j in range(NB):
                    nc.tensor.matmul(
                        o_tiles[j][:], lhsT=y2_sb[:, t, :],
                        rhs=htt[:, j * NBLK:(j + 1) * NBLK],
                        start=(t == 0), stop=(t == ET - 1),
                    )
            for j in range(NB):
                s1 = outp.tile([F, NBLK], f32, tag="s1")
                nc.scalar.activation(out=s1[:], in_=o_tiles[j][:], func=Copy)
                for c in range(NBLK // P):
                    tix = j * (NBLK // P) + c
                    t2 = ps_small.tile([P, F], f32, tag="tp")
                    nc.tensor.transpose(t2[:], s1[:, c * P:(c + 1) * P], ident[:])
                    osb = outp.tile([P, F], f32, tag="osb")
                    nc.scalar.activation(
                        out=osb[:], in_=t2[:], func=Copy,
                        scale=dv_all[:, tix:tix + 1],
                    )
                    nc.vector.tensor_tensor(
                        out=osb[:], in0=osb[:], in1=bias_sb[:], op=add
                    )
                    nc.sync.dma_start(out[tix * P:(tix + 1) * P, :], osb[:])

    nc.finalize()
    return nc


def _get_program():
    if "nc" not in _prog_cache:
        _prog_cache["nc"] = _build_program()
    return _prog_cache["nc"]


def make_in_maps(x, H, W, weight, bias):
    x = np.asarray(x, dtype=np.float32)
    H = np.asarray(H, dtype=np.float32)
    W = np.asarray(W, dtype=np.float32)
    weight = np.asarray(weight, dtype=np.float32)
    bias = np.asarray(bias, dtype=np.float32)

    H_bf = H.astype(ml_dtypes.bfloat16)
    wstr = np.ascontiguousarray(W.reshape(ET, P).T.astype(np.float32))
    biasb = np.ascontiguousarray(np.tile(bias[None, :], (P, 1)))
    wmat = np.ascontiguousarray(weight)

    in_maps = []
    for c in range(NCORES):
        hs = H_bf[c * NL:(c + 1) * NL, :]
        in_maps.append({
            "h": np.ascontiguousarray(hs),
            "ht": np.ascontiguousarray(hs.T),
            "xt": np.ascontiguousarray(x[c * NL:(c + 1) * NL, :].T),
            "wmat": wmat,
            "wstr": wstr,
            "biasb": biasb,
        })
    return in_maps


def run(x, H, W, weight, bias, trace=False, **kw):
    from concourse.bass_utils import run_bass_kernel_spmd

    nc = _get_program()
    in_maps = make_in_maps(x, H, W, weight, bias)
    res = run_bass_kernel_spmd(nc, in_maps, list(range(NCORES)), trace=trace, **kw)
    out = np.concatenate(
        [res.results[c]["out"] for c in range(NCORES)], axis=0
    ).astype(np.float32)
    return out, res


def kernel(x, H, W, weight, bias):
    out, _ = run(x, H, W, weight, bias, trace=False)
    return out



# revision 9
# speedup vs baseline: 1.5239x; 1.5239x over previous
"""HGNN conv kernel for Trainium2, 8 NeuronCores.

out = dv ⊙ (H @ (W·de ⊙ (H^T @ (dv ⊙ (x@weight))))) + bias
  dv = rowsum(H)^-1/2  [N], de = colsum(H)^-1  [E]
  N=16384, E=8192, F=64.

Sharding: H/x row-sharded over N across 8 cores (2048 rows each).
Host preps per-core fp8(e4m3) H shards in both layouts, paired for the
tensor engine's DoubleRow mode (two 128-row chunks interleaved at the
j level) — pure layout/precision transforms; all FLOPs run on device.
fp8 is numerically safe here: the output is dominated by a coherent DC
path through the all-positive H (validated: rel err ~1.4e-3 ≪ 2e-2).
All matmul operand pairs share one dtype (mixed-dtype matmul and
tensor_tensor_reduce are hardware faults on this part - found the hard
way). Power-of-2 scales keep the fp8 operands in normal range: weight
is pre-scaled x64 (so xs=dv*xw*64), W x16 (so y2 is stored x1024), and
the final per-row scale is dv/1024.

Device per core:
  prep: xw for all 16 n-tiles into one PSUM strip. Per n-tile rowsum
        split across DVE (front half) + ACT accum (back half), summed.
  pass1: two 4-pair groups; per 512-col e-block one PSUM bank takes the
        full DoubleRow chain (2 banks ping-pong; DVE copy/add drains)
        -> ybuf bf16; half-wise AllReduce (bf16) kicks when final.
  y2:   XBAR dma-transpose of the reduced half -> [128e, 32, 64]; de
        from the ones-column strip; ACT scales by 16*W*de -> y2 e4m3.
  pass2: stream HT pair-supertiles [128, 2, 2048]; 4 persistent PSUM
        banks accumulate out^T via DoubleRow over 32 pairs; finish via
        PE transpose + ACT dv/1024 scale + DVE bias add.
"""

import numpy as np
import ml_dtypes

N, E, F = 16384, 8192, 64
NCORES = 8
NL = N // NCORES          # 2048 rows per core
P = 128
NT = NL // P              # 16 n-tiles per core
NPAIR = NT // 2           # 8 natural pair-supertiles
ET = E // P               # 64 e-tiles
EPAIR = ET // 2           # 32 transposed pair-supertiles
EBLK = 512
NB_BLK = 16               # pass-1 e-blocks of 512
HE = E // 2               # collective half width
G = 4                     # pairs per pass-1 psum chain group
NBLK = 512
NB = NL // NBLK           # 4 pass-2 output blocks
XPAD = 80                 # padded xs/y2 row length (16-aligned)

_prog_cache = {}


def _build_program():
    import concourse.bass as bass
    import concourse.mybir as mybir
    import concourse.tile as tile
    from concourse import bacc
    from concourse.masks import make_identity

    f32 = mybir.dt.float32
    bf16 = mybir.dt.bfloat16
    f8 = mybir.dt.float8e4
    DR = mybir.MatmulPerfMode.DoubleRow
    Copy = mybir.ActivationFunctionType.Copy
    add = mybir.AluOpType.add
    mult = mybir.AluOpType.mult
    X = mybir.AxisListType.X

    nc = bacc.Bacc(
        "TRN2", target_bir_lowering=False, debug=False, num_devices=NCORES
    )
    h = nc.declare_dram_parameter("h", [NPAIR, P, 2, E], f8, isOutput=False)
    ht = nc.declare_dram_parameter("ht", [EPAIR, P, 2, NL], f8, isOutput=False)
    xt = nc.declare_dram_parameter("xt", [F, NL], f32, isOutput=False)
    wmat = nc.declare_dram_parameter("wmat", [F, F], f32, isOutput=False)
    wstr = nc.declare_dram_parameter("wstr", [P, ET], f32, isOutput=False)
    biasb = nc.declare_dram_parameter("biasb", [P, NB, F], f32, isOutput=False)
    out = nc.declare_dram_parameter("out", [NL, F], f32, isOutput=True)

    with tile.TileContext(nc) as tc:
        with (
            tc.tile_pool(name="hp", bufs=NPAIR) as hp,           # resident H pairs
            tc.tile_pool(name="htp", bufs=4) as htp,             # HT pair tiles
            tc.tile_pool(name="smallp", bufs=1) as smallp,       # persistent small
            tc.tile_pool(name="rp", bufs=4) as rp,               # rowsum temps
            tc.tile_pool(name="ybp", bufs=1) as ybp,             # ybuf halves
            tc.tile_pool(name="y2rp", bufs=1) as y2rp,           # y2 raw staging
            tc.tile_pool(name="finp", bufs=2) as finp,           # out staging
            tc.tile_pool(name="ps_xw", bufs=1, space="PSUM") as ps_xw,
            tc.tile_pool(name="ps_y", bufs=2, space="PSUM") as ps_y,
            tc.tile_pool(name="ps_o", bufs=1, space="PSUM") as ps_o,
            tc.tile_pool(name="dramp", bufs=1, space="DRAM") as dramp,
        ):
            # ---- persistent small tensors ----
            xt_sb = smallp.tile([F, NL], f32, tag="xt")
            nc.sync.dma_start(xt_sb[:], xt[:, :])
            wmat_sb = smallp.tile([F, F], f32, tag="wmat")
            nc.sync.dma_start(wmat_sb[:], wmat[:, :])
            wstr_sb = smallp.tile([P, ET], f32, tag="wstr")
            nc.sync.dma_start(wstr_sb[:], wstr[:, :])
            bias_sb = smallp.tile([P, NB, F], f32, tag="bias")
            nc.sync.dma_start(bias_sb[:], biasb[:, :, :])
            dv_all = smallp.tile([P, NT], f32, tag="dv")
            dvf = smallp.tile([P, NT], f32, tag="dvf")
            xs_sb = smallp.tile([P, NPAIR, 2, XPAD], f8, tag="xs")
            nc.gpsimd.memset(xs_sb[:, :, :, F:F + 1], 1.0)
            y2_sb = smallp.tile([P, EPAIR, 2, XPAD], f8, tag="y2")
            dumA = smallp.tile([P, E // 2], f8, tag="dumA")  # ACT rowsum sink
            ident = smallp.tile([F, F], f32, tag="ident")
            make_identity(nc, ident)

            # ---- xw = x @ weight (x64) for all n-tiles, kept in PSUM ----
            xw_all = ps_xw.tile([P, NT * F], f32, tag="xw")
            for t in range(NT):
                nc.tensor.matmul(
                    xw_all[:, t * F:(t + 1) * F],
                    lhsT=xt_sb[:, t * P:(t + 1) * P], rhs=wmat_sb[:],
                    start=True, stop=True,
                )

            h_tiles = []

            def prep_pair(s):
                h_s = hp.tile([P, 2, E], f8, tag="h")
                nc.sync.dma_start(h_s[:], h[s, :, :, :])
                h_tiles.append(h_s)
                for j in range(2):
                    t = 2 * s + j
                    # rowsum split: DVE front half, ACT-accum back half
                    r0 = rp.tile([P, 1], f32, tag="r0")
                    nc.vector.tensor_reduce(
                        out=r0[:], in_=h_s[:, j, 0:E // 2], axis=X, op=add
                    )
                    r1 = rp.tile([P, 1], f32, tag="r1")
                    nc.scalar.activation(
                        out=dumA[:], in_=h_s[:, j, E // 2:E], func=Copy,
                        accum_out=r1[:],
                    )
                    rinv = rp.tile([P, 1], f32, tag="rinv")
                    nc.vector.tensor_tensor(out=rinv[:], in0=r0[:], in1=r1[:], op=add)
                    nc.vector.reciprocal(out=rinv[:], in_=rinv[:])
                    nc.scalar.sqrt(out=dv_all[:, t:t + 1], in_=rinv[:])
                    nc.scalar.activation(
                        out=xs_sb[:, s, j, 0:F], in_=xw_all[:, t * F:(t + 1) * F],
                        func=Copy, scale=dv_all[:, t:t + 1],
                    )

            for s in range(G):
                prep_pair(s)

            # ---- pass 1 (DoubleRow chains, 2 rotating banks) ----
            ybufs = [ybp.tile([F + 1, HE], bf16, name=f"yb{q}") for q in range(2)]

            def p1_block(b, g):
                yps = ps_y.tile([F + 1, EBLK], f32, tag="yps")
                for i in range(G):
                    pr = g * G + i
                    nc.tensor.matmul(
                        yps[:], lhsT=xs_sb[:, pr, :, 0:F + 1],
                        rhs=h_tiles[pr][:, :, b * EBLK:(b + 1) * EBLK],
                        start=(i == 0), stop=(i == G - 1),
                        perf_mode=DR,
                    )
                return yps

            # group 0: pairs 0..3; prep of pairs 4..7 interleaves
            for b in range(NB_BLK):
                if b % 2 == 0 and b // 2 < G:
                    prep_pair(G + b // 2)
                yps = p1_block(b, 0)
                nc.vector.tensor_copy(
                    out=ybufs[b // 8][:, (b % 8) * EBLK:(b % 8 + 1) * EBLK],
                    in_=yps[:],
                )

            # group 1: pairs 4..7; half-wise collective kick when final
            for b in range(NB_BLK):
                yps = p1_block(b, 1)
                dst = ybufs[b // 8][:, (b % 8) * EBLK:(b % 8 + 1) * EBLK]
                nc.vector.tensor_tensor(out=dst, in0=dst, in1=yps[:], op=add)
                if b % 8 == 7:
                    q = b // 8
                    b_in = dramp.tile([F + 1, HE], bf16, name=f"bi{q}")
                    b_out = dramp.tile([F + 1, HE], bf16, name=f"bo{q}")
                    nc.sync.dma_start(b_in[:], ybufs[q][:])
                    nc.gpsimd.collective_compute(
                        "AllReduce",
                        add,
                        ins=[b_in[:].opt()],
                        outs=[b_out[:].opt()],
                        replica_groups=[list(range(NCORES))],
                    )
                    # y2 half prep: XBAR transpose + de/wde + scale
                    y2raw = y2rp.tile([P, HE // P, F], bf16, tag="y2r")
                    nc.sync.dma_start_transpose(y2raw[:], b_out[0:F, :])
                    cs = smallp.tile([P, HE // P], bf16, name=f"cs{q}")
                    nc.sync.dma_start(
                        cs[:], b_out[F, :].rearrange("(o p) -> p o", p=P)
                    )
                    de_t = smallp.tile([P, HE // P], f32, name=f"de{q}")
                    nc.vector.reciprocal(out=de_t[:], in_=cs[:])
                    wde = smallp.tile([P, HE // P], f32, name=f"wde{q}")
                    nc.vector.tensor_tensor(
                        out=wde[:], in0=de_t[:],
                        in1=wstr_sb[:, q * (HE // P):(q + 1) * (HE // P)], op=mult,
                    )
                    for c in range(HE // P):
                        o = q * (HE // P) + c
                        nc.scalar.activation(
                            out=y2_sb[:, o // 2, o % 2, 0:F], in_=y2raw[:, c, :],
                            func=Copy, scale=wde[:, c:c + 1],
                        )

            # ---- pass 2: out^T accumulation via DoubleRow over e-pairs ----
            o_tiles = [ps_o.tile([F, NBLK], f32, name=f"o{j}") for j in range(NB)]
            for u in range(EPAIR):
                htt = htp.tile([P, 2, NL], f8, tag="ht")
                nc.sync.dma_start(htt[:], ht[u, :, :, :])
                for j in range(NB):
                    nc.tensor.matmul(
                        o_tiles[j][:], lhsT=y2_sb[:, u, :, 0:F],
                        rhs=htt[:, :, j * NBLK:(j + 1) * NBLK],
                        start=(u == 0), stop=(u == EPAIR - 1),
                        perf_mode=DR,
                    )

            # ---- finish: transpose back, dv/1024 scale, bias, store ----
            nc.vector.tensor_scalar_mul(dvf[:], dv_all[:], 1.0 / 1024.0)
            for j in range(NB):
                s1 = finp.tile([F, NBLK], f32, tag="s1")
                nc.vector.tensor_copy(out=s1[:], in_=o_tiles[j][:])
                osb = finp.tile([P, NBLK // P, F], f32, tag="osb")
                for c in range(NBLK // P):
                    tp = ps_y.tile([P, F], f32, tag="yps")
                    nc.tensor.transpose(
                        tp[:], s1[:, c * P:(c + 1) * P], ident[:]
                    )
                    nc.scalar.activation(
                        out=osb[:, c, :], in_=tp[:], func=Copy,
                        scale=dvf[:, j * (NBLK // P) + c:j * (NBLK // P) + c + 1],
                    )
                nc.vector.tensor_tensor(
                    out=osb[:], in0=osb[:], in1=bias_sb[:], op=add
                )
                nc.sync.dma_start(
                    out[j * NBLK:(j + 1) * NBLK, :].rearrange(
                        "(c p) f -> p c f", p=P
                    ),
                    osb[:],
                )

    nc.finalize()
    return nc


def _get_program():
    if "nc" not in _prog_cache:
        _prog_cache["nc"] = _build_program()
    return _prog_cache["nc"]


def make_in_maps(x, H, W, weight, bias):
    x = np.asarray(x, dtype=np.float32)
    H = np.asarray(H, dtype=np.float32)
    W = np.asarray(W, dtype=np.float32)
    weight = np.asarray(weight, dtype=np.float32)
    bias = np.asarray(bias, dtype=np.float32)

    H_f8 = H.astype(ml_dtypes.float8_e4m3)
    wstr = np.ascontiguousarray((16.0 * W).reshape(ET, P).T.astype(np.float32))
    biasb = np.ascontiguousarray(
        np.tile(bias[None, None, :], (P, NB, 1)).astype(np.float32)
    )
    wmat = np.ascontiguousarray(64.0 * weight)

    in_maps = []
    for c in range(NCORES):
        hs = H_f8[c * NL:(c + 1) * NL, :]
        # natural pairs: [NPAIR, P, 2, E], (s, p, j) -> row 256 s + 128 j + p
        hpair = np.ascontiguousarray(
            hs.reshape(NPAIR, 2, P, E).transpose(0, 2, 1, 3)
        )
        # transposed pairs: [EPAIR, P, 2, NL], (u, p, j) -> col 256 u + 128 j + p
        htpair = np.ascontiguousarray(
            hs.T.reshape(EPAIR, 2, P, NL).transpose(0, 2, 1, 3)
        )
        in_maps.append({
            "h": hpair,
            "ht": htpair,
            "xt": np.ascontiguousarray(x[c * NL:(c + 1) * NL, :].T),
            "wmat": wmat,
            "wstr": wstr,
            "biasb": biasb,
        })
    return in_maps


def run(x, H, W, weight, bias, trace=False, **kw):
    from concourse.bass_utils import run_bass_kernel_spmd

    nc = _get_program()
    in_maps = make_in_maps(x, H, W, weight, bias)
    res = run_bass_kernel_spmd(nc, in_maps, list(range(NCORES)), trace=trace, **kw)
    out = np.concatenate(
        [res.results[c]["out"] for c in range(NCORES)], axis=0
    ).astype(np.float32)
    return out, res


def kernel(x, H, W, weight, bias):
    out, _ = run(x, H, W, weight, bias, trace=False)
    return out
